# revision 12
# baseline (speedup 1.0000x reference)
"""Trainium2 8-core tensor-parallel sparse-attention kernel (Bass/Tile).

Reference (SQ=2048, B=1, H=2048, NH=16, HD=128):
    x = hidden[:,0,:] @ svd_token
    w = qkv_w @ svd_token;  mixed = x @ w.T + qkv_b
    per head h: q,k rotated by svd_qk[h], v by svd_vlin[h]
    scores = qr @ kr.T / sqrt(128) causal-masked, softmax
    ctx = probs @ vr;  tsr[h] = svd_vlin[h].T @ dense_w[h]
    out = ctx @ tsr + dense_b

Key identity: mixed = x @ (qkv_w @ st).T = (x @ st.T) @ qkv_w.T = y @ qkv_w.T
 -> compute y seq-sharded (2.15 GF/core) instead of w head-sharded (6.4).

Per-core pipeline (TP over heads, 2 heads/core):
  pass1: x^T_c = st-panels.T @ hidden^T_c            (f32r)
  pass2: y^T_c = stT-panels.T @ x^T_c  -> AllGather  (f32r)
  fill:  tsr shard -> AllGather; dense_b broadcast   (overlaps AG_y)
  B2:    mixed^T = qkv_w-shard @ y^T, rank-pairs N=512 (f32r)
  C:     per head: q/k rot f32r, v rot bf16; scores^T f32r; raw exp
         (scores bounded ~15); P@V + row-sum ones-matmul in bf16;
         normalize post-PV; ctx^T -> per-head AllToAll (bf16)
  E:     out = ctx_myblock @ tsr + dense_b, bf16 (tsr cast-prefetched
         to SBUF during C)
Host only shards inputs / concatenates the 8 output row-blocks.
"""
import math

import numpy as np

import concourse.bass as bass
import concourse.mybir as mybir
import concourse.bacc as bacc
import concourse.tile as tile
from concourse import bass_utils

N_CORES = 8
SQ = 2048
H = 2048
NH = 16
HD = 128
HPC = NH // N_CORES          # heads per core = 2
QKVR = HPC * 3 * HD          # qkv rows per core = 768
SEQB = SQ // N_CORES         # seq block per core = 256
KT = H // 128                # 128-tiles over hidden = 16
MT = QKVR // 128             # qkv row tiles = 6
F32 = mybir.dt.float32
F32R = mybir.dt.float32r
BF16 = mybir.dt.bfloat16
NEG = -30000.0
SCALE = 1.0 / math.sqrt(HD)


def r(ap):
    return ap.bitcast(F32R)


def build(causal=True):
    nc = bacc.Bacc("TRN2", target_bir_lowering=False, debug=False,
                   num_devices=N_CORES)

    hT = nc.dram_tensor("hT", [H, SEQB], F32, kind="ExternalInput")
    qwT = nc.dram_tensor("qwT", [H, QKVR], F32, kind="ExternalInput")
    qbT = nc.dram_tensor("qbT", [128, MT], F32, kind="ExternalInput")
    stok = nc.dram_tensor("stok", [H, H], F32, kind="ExternalInput")
    stokT = nc.dram_tensor("stokT", [H, H], F32, kind="ExternalInput")
    sqk = nc.dram_tensor("sqk", [HPC, HD, HD], F32, kind="ExternalInput")
    svl = nc.dram_tensor("svl", [HPC, HD, HD], F32, kind="ExternalInput")
    dw = nc.dram_tensor("dw", [HPC, HD, H], F32, kind="ExternalInput")
    dbB = nc.dram_tensor("dbB", [1, H], F32, kind="ExternalInput")
    out = nc.dram_tensor("out", [SEQB, H], F32, kind="ExternalOutput")

    ones_dram = nc.inline_tensor(np.ones((128, 128), np.float32), name="ones_c")
    tb_np = np.where(
        np.arange(128)[:, None] > np.arange(896)[None, :] - 384, NEG, 0.0
    ).astype(np.float32)
    tb_dram = nc.inline_tensor(tb_np, name="triband_c")

    rg = [list(range(N_CORES))]

    with tile.TileContext(nc) as tc:
        with (
            nc.allow_low_precision(reason="f32r/bf16 for full-rate PE"),
            tc.tile_pool(name="pers", bufs=1) as pers,
            tc.tile_pool(name="dram", bufs=1, space="DRAM") as dram,
        ):
            # ---- persistent constants ----
            ones_sb = pers.tile([128, 128], F32)
            onesb_sb = pers.tile([128, 128], BF16)
            tb_sb = pers.tile([128, 896], F32)
            nc.sync.dma_start(r(ones_sb[:]), r(ones_dram[:]))
            nc.gpsimd.dma_start(onesb_sb[:], ones_dram[:])   # cast dma
            nc.sync.dma_start(tb_sb[:], tb_dram[:])
            qb_sb = pers.tile([128, MT], F32)
            nc.sync.dma_start(qb_sb[:], qbT[:])
            sqk_sb = pers.tile([128, HPC * HD], F32)
            svl_sb = pers.tile([128, HPC * HD], F32)
            for hl in range(HPC):
                nc.sync.dma_start(r(sqk_sb[:, hl * HD:(hl + 1) * HD]), r(sqk[hl]))
                nc.sync.dma_start(r(svl_sb[:, hl * HD:(hl + 1) * HD]), r(svl[hl]))
            db_sb = pers.tile([1, H], F32)
            nc.sync.dma_start(r(db_sb[:]), r(dbB[:]))

            y_in = dram.tile([H, SEQB], F32)
            y_g = dram.tile([N_CORES * H, SEQB], F32, addr_space="Shared")
            tsr_in = dram.tile([HPC * HD, H], F32)
            tsr_g = dram.tile([NH * HD, H], F32, addr_space="Shared")

            # ---- pass1: x^T_c[j,s] = sum_k st[k,j] h^T[k,s] ----
            with (
                tc.tile_pool(name="sA", bufs=2) as sA,
                tc.tile_pool(name="pA", bufs=3, space="PSUM") as pA,
            ):
                xc_sb = sA.tile([128, KT * SEQB], F32, tag="xc", bufs=1)
                hT_sb = sA.tile([128, KT * SEQB], F32, tag="hT", bufs=1)
                nc.sync.dma_start(
                    r(hT_sb[:].rearrange("p (k s) -> p k s", k=KT)),
                    r(hT.rearrange("(k p) s -> p k s", p=128)))
                for j in range(KT):
                    stc = sA.tile([128, KT * 128], F32, tag="stc", bufs=3)
                    nc.sync.dma_start(
                        r(stc[:].rearrange("p (k j) -> p k j", k=KT)),
                        r(stok[:, j * 128:(j + 1) * 128]
                          .rearrange("(k p) j -> p k j", p=128)))
                    xp = pA.tile([128, SEQB], F32, tag="xp")
                    for k in range(KT):
                        nc.tensor.matmul(
                            xp[:], r(stc[:, k * 128:(k + 1) * 128]),
                            r(hT_sb[:, k * SEQB:(k + 1) * SEQB]),
                            start=(k == 0), stop=(k == KT - 1))
                    nc.vector.tensor_copy(
                        r(xc_sb[:, j * SEQB:(j + 1) * SEQB]), xp[:])

                # ---- pass2: y^T_c[j,s] = sum_m stT[m,j] x^T_c[m,s] ----
                for j in range(KT):
                    stc2 = sA.tile([128, KT * 128], F32, tag="stc", bufs=3)
                    nc.sync.dma_start(
                        r(stc2[:].rearrange("p (k j) -> p k j", k=KT)),
                        r(stokT[:, j * 128:(j + 1) * 128]
                          .rearrange("(k p) j -> p k j", p=128)))
                    yp = pA.tile([128, SEQB], F32, tag="xp")
                    for m in range(KT):
                        nc.tensor.matmul(
                            yp[:], r(stc2[:, m * 128:(m + 1) * 128]),
                            r(xc_sb[:, m * SEQB:(m + 1) * SEQB]),
                            start=(m == 0), stop=(m == KT - 1))
                    ys = sA.tile([128, SEQB], F32, tag="ys")
                    nc.vector.tensor_copy(ys[:], yp[:])
                    nc.sync.dma_start(y_in[j * 128:(j + 1) * 128, :], ys[:])
            nc.gpsimd.collective_compute(
                "AllGather", mybir.AluOpType.bypass, replica_groups=rg,
                ins=[y_in[:].opt()], outs=[y_g[:].opt()])

            # ---- AG_y fill: tsr shard + AG, dense_b broadcast ----
            bb_sb = pers.tile([128, H], BF16)
            with (
                tc.tile_pool(name="s0", bufs=2) as s0,
                tc.tile_pool(name="p0", bufs=2, space="PSUM") as p0,
            ):
                dw_sb = s0.tile([128, HPC * H], F32, tag="dwt", bufs=1)
                for hl in range(HPC):
                    nc.sync.dma_start(r(dw_sb[:, hl * H:(hl + 1) * H]), r(dw[hl]))
                for hl in range(HPC):
                    tsr_sb = s0.tile([128, H], F32, tag="tsr")
                    for n in range(4):
                        tp = p0.tile([128, 512], F32, tag="t0p")
                        nc.tensor.matmul(
                            tp[:], r(svl_sb[:, hl * HD:(hl + 1) * HD]),
                            r(dw_sb[:, hl * H + n * 512: hl * H + (n + 1) * 512]),
                            start=True, stop=True)
                        nc.vector.tensor_copy(
                            tsr_sb[:, n * 512:(n + 1) * 512], tp[:])
                    nc.sync.dma_start(tsr_in[hl * HD:(hl + 1) * HD, :], tsr_sb[:])
                for n in range(4):
                    bp = p0.tile([128, 512], F32, tag="t0p")
                    nc.tensor.matmul(bp[:], r(ones_sb[0:1, :]),
                                     r(db_sb[:, n * 512:(n + 1) * 512]),
                                     start=True, stop=True)
                    nc.vector.tensor_copy(bb_sb[:, n * 512:(n + 1) * 512], bp[:])
            nc.gpsimd.collective_compute(
                "AllGather", mybir.AluOpType.bypass, replica_groups=rg,
                ins=[tsr_in[:].opt()], outs=[tsr_g[:].opt()])

            # ---- B2: mixed^T = qw_shard @ y^T, rank-pairs (N=512) ----
            mid = tc.alloc_tile_pool(name="mid", bufs=1)
            mixT = mid.tile([128, MT * SQ], F32, name="mixT")
            tsrb_sb = mid.tile([128, KT * H], BF16, name="tsrb_sb")
            with (
                tc.tile_pool(name="sB", bufs=2) as sB,
                tc.tile_pool(name="pB", bufs=6, space="PSUM") as pB,
            ):
                qwT_sb = sB.tile([128, KT * QKVR], F32, tag="qwT", bufs=1)
                nc.sync.dma_start(
                    r(qwT_sb[:].rearrange("p (k q) -> p k q", k=KT)),
                    r(qwT.rearrange("(k p) q -> p k q", p=128)))
                for rp in range(N_CORES // 2):
                    mps = [pB.tile([128, 512], F32, tag="mp", name=f"mp{rp}_{i}") for i in range(MT)]
                    for k in range(KT):
                        yg_sb = sB.tile([128, 512], F32, tag="yg", bufs=4)
                        for half in range(2):
                            rb = rp * 2 + half
                            nc.sync.dma_start(
                                r(yg_sb[:, half * SEQB:(half + 1) * SEQB]),
                                r(y_g[rb * H + k * 128: rb * H + (k + 1) * 128, :]))
                        for mt in range(MT):
                            nc.tensor.matmul(
                                mps[mt][:],
                                r(qwT_sb[:, k * QKVR + mt * 128:
                                         k * QKVR + (mt + 1) * 128]),
                                r(yg_sb[:]),
                                start=(k == 0), stop=(k == KT - 1))
                    for mt in range(MT):
                        nc.vector.tensor_scalar_add(
                            r(mixT[:, mt * SQ + rp * 512: mt * SQ + (rp + 1) * 512]),
                            mps[mt][:], qb_sb[:, mt:mt + 1])

            # ---- stage C: rotations + attention per head ----
            ctx_in = [dram.tile([N_CORES, HD, SEQB], BF16, name=f"ctxin{hl}")
                      for hl in range(HPC)]
            ctx_a = [dram.tile([N_CORES, HD, SEQB], BF16, name=f"ctxa{hl}")
                     for hl in range(HPC)]
            # prefetch tsr (f32 dram -> bf16 sbuf cast-dma) during stage C
            for kt in range(KT):
                nc.gpsimd.dma_start(
                    tsrb_sb[:, kt * H:(kt + 1) * H],
                    tsr_g[kt * 128:(kt + 1) * 128, :])
            with (
                tc.tile_pool(name="sC", bufs=1) as sC,
                tc.tile_pool(name="pC", bufs=2, space="PSUM") as pC,
                tc.tile_pool(name="sD", bufs=2) as sD,
            ):
                for hl in range(HPC):
                    qrow, krow, vrow = hl * 3, hl * 3 + 1, hl * 3 + 2
                    qrotT = sC.tile([128, SQ], F32, tag="qrot", bufs=2)
                    krotT = sC.tile([128, SQ], F32, tag="krot", bufs=2)
                    vrot = sC.tile([128, SQ], BF16, tag="vrot", bufs=2)
                    for sc in range(4):
                        rp1 = pC.tile([128, 512], F32, tag="rotp")
                        nc.tensor.matmul(
                            rp1[:], r(sqk_sb[:, hl * HD:(hl + 1) * HD]),
                            r(mixT[:, qrow * SQ + sc * 512:
                                   qrow * SQ + (sc + 1) * 512]),
                            start=True, stop=True)
                        nc.scalar.activation(
                            r(qrotT[:, sc * 512:(sc + 1) * 512]), rp1[:],
                            mybir.ActivationFunctionType.Copy, scale=SCALE)
                        rp2 = pC.tile([128, 512], F32, tag="rotp")
                        nc.tensor.matmul(
                            rp2[:], r(sqk_sb[:, hl * HD:(hl + 1) * HD]),
                            r(mixT[:, krow * SQ + sc * 512:
                                   krow * SQ + (sc + 1) * 512]),
                            start=True, stop=True)
                        nc.vector.tensor_copy(
                            r(krotT[:, sc * 512:(sc + 1) * 512]), rp2[:])
                    for st in range(KT):
                        vp = pC.tile([128, 128], F32, tag="rotp")
                        nc.tensor.matmul(
                            vp[:],
                            r(mixT[:, vrow * SQ + st * 128:
                                   vrow * SQ + (st + 1) * 128]),
                            r(svl_sb[:, hl * HD:(hl + 1) * HD]),
                            start=True, stop=True)
                        nc.vector.tensor_copy(vrot[:, st * 128:(st + 1) * 128],
                                              vp[:])

                    ctxT_sb = sC.tile([128, SQ], BF16, tag="ctxT", bufs=2)
                    for rb in range(4):
                        ncb = 4 * (rb + 1) if causal else KT
                        ctp = pC.tile([128, 512], F32, tag="ctp")
                        lp = pC.tile([1, 512], F32, tag="lp", bufs=1)
                        for cb in range(ncb):
                            sp = pC.tile([128, 512], F32, tag="sp")
                            nc.tensor.matmul(
                                sp[:], r(krotT[:, cb * 128:(cb + 1) * 128]),
                                r(qrotT[:, rb * 512:(rb + 1) * 512]),
                                start=True, stop=True)
                            if causal and cb >= 4 * rb:
                                o = 384 - (cb * 128 - rb * 512)
                                nc.vector.tensor_tensor(
                                    sp[:], sp[:], tb_sb[:, o:o + 512],
                                    mybir.AluOpType.add)
                            pT = sD.tile([128, 512], BF16, tag="pT", bufs=3)
                            nc.scalar.activation(
                                pT[:], sp[:], mybir.ActivationFunctionType.Exp)
                            nc.tensor.matmul(
                                ctp[:], vrot[:, cb * 128:(cb + 1) * 128], pT[:],
                                start=(cb == 0), stop=(cb == ncb - 1))
                            nc.tensor.matmul(
                                lp[:], onesb_sb[:, 0:1], pT[:],
                                start=(cb == 0), stop=(cb == ncb - 1))
                        linv = sD.tile([1, 512], F32, tag="linv")
                        nc.vector.reciprocal(r(linv[:]), lp[:])
                        lbp = pC.tile([128, 512], F32, tag="lbp", bufs=1)
                        nc.tensor.matmul(lbp[:], r(ones_sb[0:1, :]), r(linv[:]),
                                         start=True, stop=True)
                        lb_sb = sD.tile([128, 512], F32, tag="lb")
                        nc.vector.tensor_copy(lb_sb[:], lbp[:])
                        nc.vector.tensor_tensor(
                            ctxT_sb[:, rb * 512:(rb + 1) * 512], ctp[:], lb_sb[:],
                            mybir.AluOpType.mult)
                    for b in range(N_CORES):
                        nc.sync.dma_start(
                            ctx_in[hl][b, :, :],
                            ctxT_sb[:, b * SEQB:(b + 1) * SEQB])
                    nc.gpsimd.collective_compute(
                        "AllToAll", mybir.AluOpType.bypass, replica_groups=rg,
                        ins=[ctx_in[hl][:].opt()], outs=[ctx_a[hl][:].opt()])

            # ---- stage E: out = ctx_myblock @ tsr + dense_b (bf16) ----
            with (
                tc.tile_pool(name="sE", bufs=2) as sE,
                tc.tile_pool(name="pE", bufs=4, space="PSUM") as pE,
            ):
                ctxa_sb = sE.tile([128, KT * SEQB], BF16, tag="ctxa", bufs=1)
                for kt in range(KT):
                    nc.sync.dma_start(
                        ctxa_sb[:, kt * SEQB:(kt + 1) * SEQB],
                        ctx_a[kt % HPC][kt // HPC, :, :])
                for n in range(4):
                    for mt in range(2):
                        op = pE.tile([128, 512], F32, tag="op")
                        for kt in range(KT):
                            nc.tensor.matmul(
                                op[:],
                                ctxa_sb[:, kt * SEQB + mt * 128:
                                        kt * SEQB + (mt + 1) * 128],
                                tsrb_sb[:, kt * H + n * 512:
                                        kt * H + (n + 1) * 512],
                                start=(kt == 0), stop=(kt == KT - 1))
                        os_ = sE.tile([128, 512], F32, tag="os")
                        nc.vector.tensor_tensor(
                            os_[:], op[:], bb_sb[:, n * 512:(n + 1) * 512],
                            mybir.AluOpType.add)
                        nc.sync.dma_start(
                            out[mt * 128:(mt + 1) * 128, n * 512:(n + 1) * 512],
                            os_[:])
            mid.release()
    nc.compile()
    return nc


_CAUSAL_MASK = None


def _is_causal(mask):
    global _CAUSAL_MASK
    m = np.asarray(mask).reshape(SQ, SQ)
    if _CAUSAL_MASK is None:
        _CAUSAL_MASK = np.triu(np.ones((SQ, SQ), dtype=bool), k=1)
    return np.array_equal(m, _CAUSAL_MASK)


def make_in_maps(inputs):
    hidden_states = np.asarray(inputs["hidden_states"], np.float32)
    qkv_w = np.asarray(inputs["qkv_w"], np.float32)
    qkv_b = np.asarray(inputs["qkv_b"], np.float32)
    svd_token = np.ascontiguousarray(np.asarray(inputs["svd_token"], np.float32))
    svd_tokenT = np.ascontiguousarray(svd_token.T)
    svd_qk = np.asarray(inputs["svd_qk"], np.float32)
    svd_vlin = np.asarray(inputs["svd_vlin"], np.float32)
    dense_w = np.asarray(inputs["dense_w"], np.float32)
    dense_b = np.asarray(inputs["dense_b"], np.float32)

    hTf = np.ascontiguousarray(hidden_states[:, 0, :].T)        # [H, SQ]
    qwTf = np.ascontiguousarray(qkv_w.T)                        # [H, 3H]
    in_maps = []
    for c in range(N_CORES):
        h0 = c * HPC
        rows = slice(c * QKVR, (c + 1) * QKVR)
        in_maps.append({
            "hT": np.ascontiguousarray(hTf[:, c * SEQB:(c + 1) * SEQB]),
            "qwT": np.ascontiguousarray(qwTf[:, rows]),
            "qbT": np.ascontiguousarray(qkv_b[rows].reshape(MT, 128).T),
            "stok": svd_token,
            "stokT": svd_tokenT,
            "sqk": np.ascontiguousarray(svd_qk[h0:h0 + HPC]),
            "svl": np.ascontiguousarray(svd_vlin[h0:h0 + HPC]),
            "dw": np.ascontiguousarray(dense_w[h0:h0 + HPC]),
            "dbB": np.ascontiguousarray(dense_b.reshape(1, H)),
        })
    return in_maps


def kernel(hidden_states, attention_mask, qkv_w, qkv_b, svd_token,
           svd_qk, svd_vlin, dense_w, dense_b):
    causal = _is_causal(attention_mask)
    if not causal:
        assert not np.asarray(attention_mask).any(), \
            "kernel supports causal or empty attention_mask"

    nc = build(causal=causal)
    in_maps = make_in_maps({
        "hidden_states": hidden_states, "qkv_w": qkv_w, "qkv_b": qkv_b,
        "svd_token": svd_token, "svd_qk": svd_qk, "svd_vlin": svd_vlin,
        "dense_w": dense_w, "dense_b": dense_b,
    })
    res = bass_utils.run_bass_kernel_spmd(
        nc, in_maps, core_ids=list(range(N_CORES)), trace=False)
    full = np.concatenate([res.results[c]["out"] for c in range(N_CORES)], axis=0)
    return full.reshape(SQ, 1, H)


# revision 14
# speedup vs baseline: 1.0895x; 1.0895x over previous
"""Trainium2 8-core tensor-parallel sparse-attention kernel (Bass/Tile).

Reference (SQ=2048, B=1, H=2048, NH=16, HD=128):
    x = hidden[:,0,:] @ svd_token
    w = qkv_w @ svd_token;  mixed = x @ w.T + qkv_b
    per head h: q,k rotated by svd_qk[h], v by svd_vlin[h]
    scores = qr @ kr.T / sqrt(128) causal-masked, softmax
    ctx = probs @ vr;  tsr[h] = svd_vlin[h].T @ dense_w[h]
    out = ctx @ tsr + dense_b

Key identity: mixed = x @ (qkv_w @ st).T = (x @ st.T) @ qkv_w.T = y @ qkv_w.T
 -> compute y seq-sharded (2.15 GF/core) instead of w head-sharded (6.4).

Per-core pipeline (TP over heads, 2 heads/core):
  pass1: x^T_c = st-panels.T @ hidden^T_c            (f32r)
  pass2: y^T_c = stT-panels.T @ x^T_c  -> AllGather  (f32r)
  fill:  tsr shard -> AllGather; dense_b broadcast   (overlaps AG_y)
  B2:    mixed^T = qkv_w-shard @ y^T, rank-pairs N=512 (f32r)
  C:     per head: q/k rot f32r, v rot bf16; scores^T f32r; raw exp
         (scores bounded ~15); P@V + row-sum ones-matmul in bf16;
         normalize post-PV; ctx^T -> per-head AllToAll (bf16)
  E:     out = ctx_myblock @ tsr + dense_b, bf16 (tsr cast-prefetched
         to SBUF during C)
Host only shards inputs / concatenates the 8 output row-blocks.
"""
import math

import numpy as np

import concourse.bass as bass
import concourse.mybir as mybir
import concourse.bacc as bacc
import concourse.tile as tile
from concourse import bass_utils

N_CORES = 8
SQ = 2048
H = 2048
NH = 16
HD = 128
HPC = NH // N_CORES          # heads per core = 2
QKVR = HPC * 3 * HD          # qkv rows per core = 768
SEQB = SQ // N_CORES         # seq block per core = 256
KT = H // 128                # 128-tiles over hidden = 16
MT = QKVR // 128             # qkv row tiles = 6
F32 = mybir.dt.float32
F32R = mybir.dt.float32r
BF16 = mybir.dt.bfloat16
NEG = -30000.0
SCALE = 1.0 / math.sqrt(HD)


def r(ap):
    return ap.bitcast(F32R)


def build(causal=True):
    nc = bacc.Bacc("TRN2", target_bir_lowering=False, debug=False,
                   num_devices=N_CORES)

    hT = nc.dram_tensor("hT", [H, SEQB], F32, kind="ExternalInput")
    qwT = nc.dram_tensor("qwT", [H, QKVR], F32, kind="ExternalInput")
    qbT = nc.dram_tensor("qbT", [128, MT], F32, kind="ExternalInput")
    stok = nc.dram_tensor("stok", [KT, 128, KT * 128], F32, kind="ExternalInput")
    stokT = nc.dram_tensor("stokT", [KT, 128, KT * 128], F32, kind="ExternalInput")
    sqk = nc.dram_tensor("sqk", [HPC, HD, HD], F32, kind="ExternalInput")
    svl = nc.dram_tensor("svl", [HPC, HD, HD], F32, kind="ExternalInput")
    dw = nc.dram_tensor("dw", [HPC, HD, H], F32, kind="ExternalInput")
    dbB = nc.dram_tensor("dbB", [1, H], F32, kind="ExternalInput")
    out = nc.dram_tensor("out", [SEQB, H], F32, kind="ExternalOutput")

    ones_dram = nc.inline_tensor(np.ones((128, 128), np.float32), name="ones_c")
    tb_np = np.where(
        np.arange(128)[:, None] > np.arange(896)[None, :] - 384, NEG, 0.0
    ).astype(np.float32)
    tb_dram = nc.inline_tensor(tb_np, name="triband_c")

    rg = [list(range(N_CORES))]

    with tile.TileContext(nc) as tc:
        with (
            nc.allow_low_precision(reason="f32r/bf16 for full-rate PE"),
            tc.tile_pool(name="pers", bufs=1) as pers,
            tc.tile_pool(name="dram", bufs=1, space="DRAM") as dram,
        ):
            # ---- persistent constants ----
            ones_sb = pers.tile([128, 128], F32)
            onesb_sb = pers.tile([128, 128], BF16)
            tb_sb = pers.tile([128, 896], F32)
            nc.sync.dma_start(r(ones_sb[:]), r(ones_dram[:]))
            nc.gpsimd.dma_start(onesb_sb[:], ones_dram[:])   # cast dma
            nc.sync.dma_start(tb_sb[:], tb_dram[:])
            qb_sb = pers.tile([128, MT], F32)
            nc.sync.dma_start(qb_sb[:], qbT[:])
            sqk_sb = pers.tile([128, HPC * HD], F32)
            svl_sb = pers.tile([128, HPC * HD], F32)
            for hl in range(HPC):
                nc.sync.dma_start(r(sqk_sb[:, hl * HD:(hl + 1) * HD]), r(sqk[hl]))
                nc.sync.dma_start(r(svl_sb[:, hl * HD:(hl + 1) * HD]), r(svl[hl]))
            db_sb = pers.tile([1, H], F32)
            nc.sync.dma_start(r(db_sb[:]), r(dbB[:]))

            y_in1 = dram.tile([H // 2, SEQB], F32)
            y_in2 = dram.tile([H // 2, SEQB], F32)
            y_g1 = dram.tile([N_CORES * H // 2, SEQB], F32, addr_space="Shared")
            y_g2 = dram.tile([N_CORES * H // 2, SEQB], F32, addr_space="Shared")
            tsr_in = dram.tile([HPC * HD, H], F32)
            tsr_g = dram.tile([NH * HD, H], F32, addr_space="Shared")

            # ---- tsr shard + AG_tsr first (rides under pass1/pass2) ----
            bb_sb = pers.tile([128, H], BF16)
            with (
                tc.tile_pool(name="s0", bufs=2) as s0,
                tc.tile_pool(name="p0", bufs=2, space="PSUM") as p0,
            ):
                dw_sb = s0.tile([128, HPC * H], F32, tag="dwt", bufs=1)
                for hl in range(HPC):
                    nc.sync.dma_start(r(dw_sb[:, hl * H:(hl + 1) * H]), r(dw[hl]))
                for hl in range(HPC):
                    tsr_sb = s0.tile([128, H], F32, tag="tsr")
                    for n in range(4):
                        tp = p0.tile([128, 512], F32, tag="t0p")
                        nc.tensor.matmul(
                            tp[:], r(svl_sb[:, hl * HD:(hl + 1) * HD]),
                            r(dw_sb[:, hl * H + n * 512: hl * H + (n + 1) * 512]),
                            start=True, stop=True)
                        nc.vector.tensor_copy(
                            tsr_sb[:, n * 512:(n + 1) * 512], tp[:])
                    nc.sync.dma_start(tsr_in[hl * HD:(hl + 1) * HD, :], tsr_sb[:])
                for n in range(4):
                    bp = p0.tile([128, 512], F32, tag="t0p")
                    nc.tensor.matmul(bp[:], r(ones_sb[0:1, :]),
                                     r(db_sb[:, n * 512:(n + 1) * 512]),
                                     start=True, stop=True)
                    nc.vector.tensor_copy(bb_sb[:, n * 512:(n + 1) * 512], bp[:])
            nc.gpsimd.collective_compute(
                "AllGather", mybir.AluOpType.bypass, replica_groups=rg,
                ins=[tsr_in[:].opt()], outs=[tsr_g[:].opt()])

            # ---- pass1: x^T_c[j,s] = sum_k st[k,j] h^T[k,s] ----
            with (
                tc.tile_pool(name="sA", bufs=2) as sA,
                tc.tile_pool(name="pA", bufs=3, space="PSUM") as pA,
            ):
                xc_sb = sA.tile([128, KT * SEQB], F32, tag="xc", bufs=1)
                hT_sb = sA.tile([128, KT * SEQB], F32, tag="hT", bufs=1)
                nc.sync.dma_start(
                    r(hT_sb[:].rearrange("p (k s) -> p k s", k=KT)),
                    r(hT.rearrange("(k p) s -> p k s", p=128)))
                for j in range(KT):
                    stc = sA.tile([128, KT * 128], F32, tag="stc", bufs=3)
                    nc.sync.dma_start(r(stc[:]), r(stok[j]))
                    xp = pA.tile([128, SEQB], F32, tag="xp")
                    for k in range(KT):
                        nc.tensor.matmul(
                            xp[:], r(stc[:, k * 128:(k + 1) * 128]),
                            r(hT_sb[:, k * SEQB:(k + 1) * SEQB]),
                            start=(k == 0), stop=(k == KT - 1))
                    nc.vector.tensor_copy(
                        r(xc_sb[:, j * SEQB:(j + 1) * SEQB]), xp[:])

                # ---- pass2: y^T_c[j,s] = sum_m stT[m,j] x^T_c[m,s] ----
                for j in range(KT):
                    stc2 = sA.tile([128, KT * 128], F32, tag="stc", bufs=3)
                    nc.sync.dma_start(r(stc2[:]), r(stokT[j]))
                    yp = pA.tile([128, SEQB], F32, tag="xp")
                    for m in range(KT):
                        nc.tensor.matmul(
                            yp[:], r(stc2[:, m * 128:(m + 1) * 128]),
                            r(xc_sb[:, m * SEQB:(m + 1) * SEQB]),
                            start=(m == 0), stop=(m == KT - 1))
                    ys = sA.tile([128, SEQB], F32, tag="ys")
                    nc.vector.tensor_copy(ys[:], yp[:])
                    if j < KT // 2:
                        nc.sync.dma_start(y_in1[j * 128:(j + 1) * 128, :], ys[:])
                    else:
                        nc.sync.dma_start(
                            y_in2[(j - KT // 2) * 128:(j - KT // 2 + 1) * 128, :],
                            ys[:])
                    if j == KT // 2 - 1:
                        nc.gpsimd.collective_compute(
                            "AllGather", mybir.AluOpType.bypass, replica_groups=rg,
                            ins=[y_in1[:].opt()], outs=[y_g1[:].opt()])
            nc.gpsimd.collective_compute(
                "AllGather", mybir.AluOpType.bypass, replica_groups=rg,
                ins=[y_in2[:].opt()], outs=[y_g2[:].opt()])

            # ---- B2: mixed^T = qw_shard @ y^T, rank-pairs (N=512) ----
            mid = tc.alloc_tile_pool(name="mid", bufs=1)
            mixT = mid.tile([128, MT * SQ], F32, name="mixT")
            tsrb_sb = mid.tile([128, KT * H], BF16, name="tsrb_sb")
            with (
                tc.tile_pool(name="sB", bufs=2) as sB,
                tc.tile_pool(name="pB", bufs=6, space="PSUM") as pB,
            ):
                qwT_sb = sB.tile([128, KT * QKVR], F32, tag="qwT", bufs=1)
                nc.sync.dma_start(
                    r(qwT_sb[:].rearrange("p (k q) -> p k q", k=KT)),
                    r(qwT.rearrange("(k p) q -> p k q", p=128)))
                for rp in range(N_CORES // 2):
                    mps = [pB.tile([128, 512], F32, tag="mp", name=f"mp{rp}_{i}") for i in range(MT)]
                    for k in range(KT):
                        yg_sb = sB.tile([128, 512], F32, tag="yg", bufs=4)
                        ysrc = y_g1 if k < KT // 2 else y_g2
                        kk = k if k < KT // 2 else k - KT // 2
                        for half in range(2):
                            rb = rp * 2 + half
                            nc.sync.dma_start(
                                r(yg_sb[:, half * SEQB:(half + 1) * SEQB]),
                                r(ysrc[rb * H // 2 + kk * 128:
                                       rb * H // 2 + (kk + 1) * 128, :]))
                        for mt in range(MT):
                            nc.tensor.matmul(
                                mps[mt][:],
                                r(qwT_sb[:, k * QKVR + mt * 128:
                                         k * QKVR + (mt + 1) * 128]),
                                r(yg_sb[:]),
                                start=(k == 0), stop=(k == KT - 1))
                    for mt in range(MT):
                        nc.vector.tensor_scalar_add(
                            r(mixT[:, mt * SQ + rp * 512: mt * SQ + (rp + 1) * 512]),
                            mps[mt][:], qb_sb[:, mt:mt + 1])

            # ---- stage C: rotations + attention per head ----
            ctx_in = [dram.tile([N_CORES, HD, SEQB], BF16, name=f"ctxin{hl}")
                      for hl in range(HPC)]
            ctx_a = [dram.tile([N_CORES, HD, SEQB], BF16, name=f"ctxa{hl}")
                     for hl in range(HPC)]
            # prefetch tsr (f32 dram -> bf16 sbuf cast-dma) during stage C
            for kt in range(KT):
                nc.gpsimd.dma_start(
                    tsrb_sb[:, kt * H:(kt + 1) * H],
                    tsr_g[kt * 128:(kt + 1) * 128, :])
            with (
                tc.tile_pool(name="sC", bufs=1) as sC,
                tc.tile_pool(name="pC", bufs=2, space="PSUM") as pC,
                tc.tile_pool(name="sD", bufs=2) as sD,
            ):
                for hl in range(HPC):
                    qrow, krow, vrow = hl * 3, hl * 3 + 1, hl * 3 + 2
                    qrotT = sC.tile([128, SQ], F32, tag="qrot", bufs=2)
                    krotT = sC.tile([128, SQ], F32, tag="krot", bufs=2)
                    vrot = sC.tile([128, SQ], BF16, tag="vrot", bufs=2)
                    for sc in range(4):
                        rp1 = pC.tile([128, 512], F32, tag="rotp")
                        nc.tensor.matmul(
                            rp1[:], r(sqk_sb[:, hl * HD:(hl + 1) * HD]),
                            r(mixT[:, qrow * SQ + sc * 512:
                                   qrow * SQ + (sc + 1) * 512]),
                            start=True, stop=True)
                        nc.scalar.activation(
                            r(qrotT[:, sc * 512:(sc + 1) * 512]), rp1[:],
                            mybir.ActivationFunctionType.Copy, scale=SCALE)
                        rp2 = pC.tile([128, 512], F32, tag="rotp")
                        nc.tensor.matmul(
                            rp2[:], r(sqk_sb[:, hl * HD:(hl + 1) * HD]),
                            r(mixT[:, krow * SQ + sc * 512:
                                   krow * SQ + (sc + 1) * 512]),
                            start=True, stop=True)
                        nc.vector.tensor_copy(
                            r(krotT[:, sc * 512:(sc + 1) * 512]), rp2[:])
                    for st in range(KT):
                        vp = pC.tile([128, 128], F32, tag="rotp")
                        nc.tensor.matmul(
                            vp[:],
                            r(mixT[:, vrow * SQ + st * 128:
                                   vrow * SQ + (st + 1) * 128]),
                            r(svl_sb[:, hl * HD:(hl + 1) * HD]),
                            start=True, stop=True)
                        nc.vector.tensor_copy(vrot[:, st * 128:(st + 1) * 128],
                                              vp[:])

                    ctxT_sb = sC.tile([128, SQ], BF16, tag="ctxT", bufs=2)
                    for rb in range(4):
                        ncb = 4 * (rb + 1) if causal else KT
                        ctp = pC.tile([128, 512], F32, tag="ctp")
                        lp = pC.tile([1, 512], F32, tag="lp", bufs=1)
                        for cb in range(ncb):
                            sp = pC.tile([128, 512], F32, tag="sp")
                            nc.tensor.matmul(
                                sp[:], r(krotT[:, cb * 128:(cb + 1) * 128]),
                                r(qrotT[:, rb * 512:(rb + 1) * 512]),
                                start=True, stop=True)
                            if causal and cb >= 4 * rb:
                                o = 384 - (cb * 128 - rb * 512)
                                nc.vector.tensor_tensor(
                                    sp[:], sp[:], tb_sb[:, o:o + 512],
                                    mybir.AluOpType.add)
                            pT = sD.tile([128, 512], BF16, tag="pT", bufs=3)
                            nc.scalar.activation(
                                pT[:], sp[:], mybir.ActivationFunctionType.Exp)
                            nc.tensor.matmul(
                                ctp[:], vrot[:, cb * 128:(cb + 1) * 128], pT[:],
                                start=(cb == 0), stop=(cb == ncb - 1))
                            nc.tensor.matmul(
                                lp[:], onesb_sb[:, 0:1], pT[:],
                                start=(cb == 0), stop=(cb == ncb - 1))
                        linv = sD.tile([1, 512], F32, tag="linv")
                        nc.vector.reciprocal(r(linv[:]), lp[:])
                        lbp = pC.tile([128, 512], F32, tag="lbp", bufs=1)
                        nc.tensor.matmul(lbp[:], r(ones_sb[0:1, :]), r(linv[:]),
                                         start=True, stop=True)
                        lb_sb = sD.tile([128, 512], F32, tag="lb")
                        nc.vector.tensor_copy(lb_sb[:], lbp[:])
                        nc.vector.tensor_tensor(
                            ctxT_sb[:, rb * 512:(rb + 1) * 512], ctp[:], lb_sb[:],
                            mybir.AluOpType.mult)
                    for b in range(N_CORES):
                        nc.sync.dma_start(
                            ctx_in[hl][b, :, :],
                            ctxT_sb[:, b * SEQB:(b + 1) * SEQB])
                    nc.gpsimd.collective_compute(
                        "AllToAll", mybir.AluOpType.bypass, replica_groups=rg,
                        ins=[ctx_in[hl][:].opt()], outs=[ctx_a[hl][:].opt()])

            # ---- stage E: out = ctx_myblock @ tsr + dense_b (bf16) ----
            with (
                tc.tile_pool(name="sE", bufs=2) as sE,
                tc.tile_pool(name="pE", bufs=4, space="PSUM") as pE,
            ):
                ctxa_sb = sE.tile([128, KT * SEQB], BF16, tag="ctxa", bufs=1)
                for kt in range(KT):
                    nc.sync.dma_start(
                        ctxa_sb[:, kt * SEQB:(kt + 1) * SEQB],
                        ctx_a[kt % HPC][kt // HPC, :, :])
                for n in range(4):
                    for mt in range(2):
                        op = pE.tile([128, 512], F32, tag="op")
                        for kt in range(KT):
                            nc.tensor.matmul(
                                op[:],
                                ctxa_sb[:, kt * SEQB + mt * 128:
                                        kt * SEQB + (mt + 1) * 128],
                                tsrb_sb[:, kt * H + n * 512:
                                        kt * H + (n + 1) * 512],
                                start=(kt == 0), stop=(kt == KT - 1))
                        os_ = sE.tile([128, 512], F32, tag="os")
                        nc.vector.tensor_tensor(
                            os_[:], op[:], bb_sb[:, n * 512:(n + 1) * 512],
                            mybir.AluOpType.add)
                        nc.sync.dma_start(
                            out[mt * 128:(mt + 1) * 128, n * 512:(n + 1) * 512],
                            os_[:])
            mid.release()
    nc.compile()
    return nc


_CAUSAL_MASK = None


def _is_causal(mask):
    global _CAUSAL_MASK
    m = np.asarray(mask).reshape(SQ, SQ)
    if _CAUSAL_MASK is None:
        _CAUSAL_MASK = np.triu(np.ones((SQ, SQ), dtype=bool), k=1)
    return np.array_equal(m, _CAUSAL_MASK)


def make_in_maps(inputs):
    hidden_states = np.asarray(inputs["hidden_states"], np.float32)
    qkv_w = np.asarray(inputs["qkv_w"], np.float32)
    qkv_b = np.asarray(inputs["qkv_b"], np.float32)
    svd_token = np.asarray(inputs["svd_token"], np.float32)
    svd_tokenT = np.ascontiguousarray(svd_token.T)
    # panel j: [p, (k jj)] = st[k*128+p, j*128+jj] -> transpose(2,1,0,3)
    stok_t = np.ascontiguousarray(
        svd_token.reshape(KT, 128, KT, 128).transpose(2, 1, 0, 3)
        .reshape(KT, 128, KT * 128))
    stokT_t = np.ascontiguousarray(
        svd_tokenT.reshape(KT, 128, KT, 128).transpose(2, 1, 0, 3)
        .reshape(KT, 128, KT * 128))
    svd_qk = np.asarray(inputs["svd_qk"], np.float32)
    svd_vlin = np.asarray(inputs["svd_vlin"], np.float32)
    dense_w = np.asarray(inputs["dense_w"], np.float32)
    dense_b = np.asarray(inputs["dense_b"], np.float32)

    hTf = np.ascontiguousarray(hidden_states[:, 0, :].T)        # [H, SQ]
    qwTf = np.ascontiguousarray(qkv_w.T)                        # [H, 3H]
    in_maps = []
    for c in range(N_CORES):
        h0 = c * HPC
        rows = slice(c * QKVR, (c + 1) * QKVR)
        in_maps.append({
            "hT": np.ascontiguousarray(hTf[:, c * SEQB:(c + 1) * SEQB]),
            "qwT": np.ascontiguousarray(qwTf[:, rows]),
            "qbT": np.ascontiguousarray(qkv_b[rows].reshape(MT, 128).T),
            "stok": stok_t,
            "stokT": stokT_t,
            "sqk": np.ascontiguousarray(svd_qk[h0:h0 + HPC]),
            "svl": np.ascontiguousarray(svd_vlin[h0:h0 + HPC]),
            "dw": np.ascontiguousarray(dense_w[h0:h0 + HPC]),
            "dbB": np.ascontiguousarray(dense_b.reshape(1, H)),
        })
    return in_maps


def kernel(hidden_states, attention_mask, qkv_w, qkv_b, svd_token,
           svd_qk, svd_vlin, dense_w, dense_b):
    causal = _is_causal(attention_mask)
    if not causal:
        assert not np.asarray(attention_mask).any(), \
            "kernel supports causal or empty attention_mask"

    nc = build(causal=causal)
    in_maps = make_in_maps({
        "hidden_states": hidden_states, "qkv_w": qkv_w, "qkv_b": qkv_b,
        "svd_token": svd_token, "svd_qk": svd_qk, "svd_vlin": svd_vlin,
        "dense_w": dense_w, "dense_b": dense_b,
    })
    res = bass_utils.run_bass_kernel_spmd(
        nc, in_maps, core_ids=list(range(N_CORES)), trace=False)
    full = np.concatenate([res.results[c]["out"] for c in range(N_CORES)], axis=0)
    return full.reshape(SQ, 1, H)


# revision 15
# speedup vs baseline: 1.1685x; 1.0725x over previous
"""Trainium2 8-core tensor-parallel sparse-attention kernel (Bass/Tile).

Reference (SQ=2048, B=1, H=2048, NH=16, HD=128):
    x = hidden[:,0,:] @ svd_token
    w = qkv_w @ svd_token;  mixed = x @ w.T + qkv_b
    per head h: q,k rotated by svd_qk[h], v by svd_vlin[h]
    scores = qr @ kr.T / sqrt(128) causal-masked, softmax
    ctx = probs @ vr;  tsr[h] = svd_vlin[h].T @ dense_w[h]
    out = ctx @ tsr + dense_b

Key identity: mixed = x @ (qkv_w @ st).T = (x @ st.T) @ qkv_w.T = y @ qkv_w.T
 -> compute y seq-sharded (2.15 GF/core) instead of w head-sharded (6.4).

Per-core pipeline (TP over heads, 2 heads/core):
  pass1: x^T_c = st-panels.T @ hidden^T_c            (f32r)
  pass2: y^T_c = stT-panels.T @ x^T_c  -> AllGather  (f32r)
  fill:  tsr shard -> AllGather; dense_b broadcast   (overlaps AG_y)
  B2:    mixed^T = qkv_w-shard @ y^T, rank-pairs N=512 (f32r)
  C:     per head: q/k rot f32r, v rot bf16; scores^T f32r; raw exp
         (scores bounded ~15); P@V + row-sum ones-matmul in bf16;
         normalize post-PV; ctx^T -> per-head AllToAll (bf16)
  E:     out = ctx_myblock @ tsr + dense_b, bf16 (tsr cast-prefetched
         to SBUF during C)
Host only shards inputs / concatenates the 8 output row-blocks.
"""
import math

import numpy as np

import concourse.bass as bass
import concourse.mybir as mybir
import concourse.bacc as bacc
import concourse.tile as tile
from concourse import bass_utils

N_CORES = 8
SQ = 2048
H = 2048
NH = 16
HD = 128
HPC = NH // N_CORES          # heads per core = 2
QKVR = HPC * 3 * HD          # qkv rows per core = 768
SEQB = SQ // N_CORES         # seq block per core = 256
KT = H // 128                # 128-tiles over hidden = 16
MT = QKVR // 128             # qkv row tiles = 6
F32 = mybir.dt.float32
F32R = mybir.dt.float32r
BF16 = mybir.dt.bfloat16
NEG = -30000.0
SCALE = 1.0 / math.sqrt(HD)


def r(ap):
    return ap.bitcast(F32R)


def build(causal=True):
    nc = bacc.Bacc("TRN2", target_bir_lowering=False, debug=False,
                   num_devices=N_CORES)

    hT = nc.dram_tensor("hT", [H, SEQB], F32, kind="ExternalInput")
    qwT = nc.dram_tensor("qwT", [H, QKVR], F32, kind="ExternalInput")
    qbT = nc.dram_tensor("qbT", [128, MT], F32, kind="ExternalInput")
    stok = nc.dram_tensor("stok", [H, H], F32, kind="ExternalInput")
    stokT = nc.dram_tensor("stokT", [H, H], F32, kind="ExternalInput")
    sqk = nc.dram_tensor("sqk", [HPC, HD, HD], F32, kind="ExternalInput")
    svl = nc.dram_tensor("svl", [HPC, HD, HD], F32, kind="ExternalInput")
    dw = nc.dram_tensor("dw", [HPC, HD, H], F32, kind="ExternalInput")
    dbB = nc.dram_tensor("dbB", [1, H], F32, kind="ExternalInput")
    out = nc.dram_tensor("out", [SEQB, H], F32, kind="ExternalOutput")

    ones_dram = nc.inline_tensor(np.ones((128, 128), np.float32), name="ones_c")
    id_dram = nc.inline_tensor(np.eye(128, dtype=np.float32), name="id_c")
    tb_np = np.where(
        np.arange(128)[:, None] > np.arange(896)[None, :] - 384, NEG, 0.0
    ).astype(np.float32)
    tb_dram = nc.inline_tensor(tb_np, name="triband_c")

    rg = [list(range(N_CORES))]

    with tile.TileContext(nc) as tc:
        with (
            nc.allow_low_precision(reason="f32r/bf16 for full-rate PE"),
            tc.tile_pool(name="pers", bufs=1) as pers,
            tc.tile_pool(name="dram", bufs=1, space="DRAM") as dram,
        ):
            # ---- persistent constants ----
            ones_sb = pers.tile([128, 128], F32)
            onesb_sb = pers.tile([128, 128], BF16)
            tb_sb = pers.tile([128, 896], F32)
            id_sb = pers.tile([128, 128], F32)
            nc.sync.dma_start(id_sb[:], id_dram[:])
            nc.sync.dma_start(r(ones_sb[:]), r(ones_dram[:]))
            nc.gpsimd.dma_start(onesb_sb[:], ones_dram[:])   # cast dma
            nc.sync.dma_start(tb_sb[:], tb_dram[:])
            qb_sb = pers.tile([128, MT], F32)
            nc.sync.dma_start(qb_sb[:], qbT[:])
            sqk_sb = pers.tile([128, HPC * HD], F32)
            svl_sb = pers.tile([128, HPC * HD], F32)
            for hl in range(HPC):
                nc.sync.dma_start(r(sqk_sb[:, hl * HD:(hl + 1) * HD]), r(sqk[hl]))
                nc.sync.dma_start(r(svl_sb[:, hl * HD:(hl + 1) * HD]), r(svl[hl]))
            db_sb = pers.tile([1, H], F32)
            nc.sync.dma_start(r(db_sb[:]), r(dbB[:]))

            y_in1 = dram.tile([H // 2, SEQB], F32)
            y_in2 = dram.tile([H // 2, SEQB], F32)
            y_g1 = dram.tile([N_CORES * H // 2, SEQB], F32, addr_space="Shared")
            y_g2 = dram.tile([N_CORES * H // 2, SEQB], F32, addr_space="Shared")
            tsr_in = dram.tile([HPC * HD, H], BF16)
            tsr_g = dram.tile([NH * HD, H], BF16, addr_space="Shared")

            # ---- pass1/pass2 (flipped): stationary = hT/xT tiles,
            #      moving = svd_token row-panels at N=512 (LDW amortized) ----
            with (
                tc.tile_pool(name="sA", bufs=2) as sA,
                tc.tile_pool(name="pA", bufs=8, space="PSUM") as pA,
            ):
                hT_sb = sA.tile([128, KT * SEQB], F32, tag="hTt", bufs=1)
                nc.sync.dma_start(
                    r(hT_sb[:].rearrange("p (k s) -> p k s", k=KT)),
                    r(hT.rearrange("(k p) s -> p k s", p=128)))
                # pass1: x[s, j] = sum_k hT[k, s].T @ stok[k, j]
                xps = [pA.tile([128, 512], F32, tag="acc", name=f"xps{i}", bufs=8)
                       for i in range(8)]
                for k in range(KT):
                    srow = sA.tile([128, H], F32, tag="srow", bufs=3)
                    nc.sync.dma_start(r(srow[:]), r(stok[k * 128:(k + 1) * 128, :]))
                    for st in range(2):
                        for jc in range(4):
                            nc.tensor.matmul(
                                xps[st * 4 + jc][:],
                                r(hT_sb[:, k * SEQB + st * 128:
                                        k * SEQB + (st + 1) * 128]),
                                r(srow[:, jc * 512:(jc + 1) * 512]),
                                start=(k == 0), stop=(k == KT - 1))
                x_sb = sA.tile([128, 2 * H], F32, tag="xsb", bufs=1)
                for i in range(8):
                    nc.vector.tensor_copy(
                        x_sb[:, i * 512:(i + 1) * 512], xps[i][:])
                # transpose x -> xT (m on partitions)
                xT_sb = sA.tile([128, KT * SEQB], F32, tag="xTt", bufs=1)
                for m in range(KT):
                    for st in range(2):
                        tp2 = pA.tile([128, 128], F32, tag="acc",
                                      name=f"tpx{m}_{st}", bufs=8)
                        nc.tensor.transpose(
                            tp2[:],
                            x_sb[:, st * H + m * 128: st * H + (m + 1) * 128],
                            id_sb[:])
                        nc.vector.tensor_copy(
                            r(xT_sb[:, m * SEQB + st * 128:
                                    m * SEQB + (st + 1) * 128]), tp2[:])
                # pass2: y[s, j2] = sum_m xT[m, s].T @ stokT[m, j2]
                yps = [pA.tile([128, 512], F32, tag="acc", name=f"yps{i}", bufs=8)
                       for i in range(8)]
                for m in range(KT):
                    srow2 = sA.tile([128, H], F32, tag="srow", bufs=3)
                    nc.sync.dma_start(r(srow2[:]),
                                      r(stokT[m * 128:(m + 1) * 128, :]))
                    for st in range(2):
                        for jc in range(4):
                            nc.tensor.matmul(
                                yps[st * 4 + jc][:],
                                r(xT_sb[:, m * SEQB + st * 128:
                                        m * SEQB + (st + 1) * 128]),
                                r(srow2[:, jc * 512:(jc + 1) * 512]),
                                start=(m == 0), stop=(m == KT - 1))
                y_sb = sA.tile([128, 2 * H], F32, tag="xsb", bufs=1)
                for i in range(8):
                    nc.vector.tensor_copy(
                        y_sb[:, i * 512:(i + 1) * 512], yps[i][:])
                # transpose y -> yT tiles, stage + DMA out, AGs per half
                for j2 in range(KT):
                    ystg = sA.tile([128, SEQB], F32, tag="ystg", bufs=4)
                    for st in range(2):
                        tp3 = pA.tile([128, 128], F32, tag="acc",
                                      name=f"tpy{j2}_{st}", bufs=8)
                        nc.tensor.transpose(
                            tp3[:],
                            y_sb[:, st * H + j2 * 128: st * H + (j2 + 1) * 128],
                            id_sb[:])
                        nc.vector.tensor_copy(
                            ystg[:, st * 128:(st + 1) * 128], tp3[:])
                    if j2 < KT // 2:
                        nc.sync.dma_start(
                            y_in1[j2 * 128:(j2 + 1) * 128, :], ystg[:])
                    else:
                        nc.sync.dma_start(
                            y_in2[(j2 - KT // 2) * 128:(j2 - KT // 2 + 1) * 128, :],
                            ystg[:])
                    if j2 == KT // 2 - 1:
                        nc.gpsimd.collective_compute(
                            "AllGather", mybir.AluOpType.bypass, replica_groups=rg,
                            ins=[y_in1[:].opt()], outs=[y_g1[:].opt()])
            nc.gpsimd.collective_compute(
                "AllGather", mybir.AluOpType.bypass, replica_groups=rg,
                ins=[y_in2[:].opt()], outs=[y_g2[:].opt()])

            # ---- tsr shard (bf16) + AG_tsr + dense_b broadcast ----
            bb_sb = pers.tile([128, H], BF16)
            with (
                tc.tile_pool(name="s0", bufs=2) as s0,
                tc.tile_pool(name="p0", bufs=2, space="PSUM") as p0,
            ):
                dw_sb = s0.tile([128, HPC * H], F32, tag="dwt", bufs=1)
                for hl in range(HPC):
                    nc.sync.dma_start(r(dw_sb[:, hl * H:(hl + 1) * H]), r(dw[hl]))
                for hl in range(HPC):
                    tsr_sb = s0.tile([128, H], BF16, tag="tsr")
                    for n in range(4):
                        tp = p0.tile([128, 512], F32, tag="t0p")
                        nc.tensor.matmul(
                            tp[:], r(svl_sb[:, hl * HD:(hl + 1) * HD]),
                            r(dw_sb[:, hl * H + n * 512: hl * H + (n + 1) * 512]),
                            start=True, stop=True)
                        nc.vector.tensor_copy(
                            tsr_sb[:, n * 512:(n + 1) * 512], tp[:])
                    nc.sync.dma_start(tsr_in[hl * HD:(hl + 1) * HD, :], tsr_sb[:])
                for n in range(4):
                    bp = p0.tile([128, 512], F32, tag="t0p")
                    nc.tensor.matmul(bp[:], r(ones_sb[0:1, :]),
                                     r(db_sb[:, n * 512:(n + 1) * 512]),
                                     start=True, stop=True)
                    nc.vector.tensor_copy(bb_sb[:, n * 512:(n + 1) * 512], bp[:])
            nc.gpsimd.collective_compute(
                "AllGather", mybir.AluOpType.bypass, replica_groups=rg,
                ins=[tsr_in[:].opt()], outs=[tsr_g[:].opt()])

            # ---- B2: mixed^T = qw_shard @ y^T, rank-pairs (N=512) ----
            mid = tc.alloc_tile_pool(name="mid", bufs=1)
            mixT = mid.tile([128, MT * SQ], F32, name="mixT")
            tsrb_sb = mid.tile([128, KT * H], BF16, name="tsrb_sb")
            with (
                tc.tile_pool(name="sB", bufs=2) as sB,
                tc.tile_pool(name="pB", bufs=6, space="PSUM") as pB,
            ):
                qwT_sb = sB.tile([128, KT * QKVR], F32, tag="qwT", bufs=1)
                nc.sync.dma_start(
                    r(qwT_sb[:].rearrange("p (k q) -> p k q", k=KT)),
                    r(qwT.rearrange("(k p) q -> p k q", p=128)))
                for rp in range(N_CORES // 2):
                    mps = [pB.tile([128, 512], F32, tag="mp", name=f"mp{rp}_{i}") for i in range(MT)]
                    for k in range(KT):
                        yg_sb = sB.tile([128, 512], F32, tag="yg", bufs=4)
                        ysrc = y_g1 if k < KT // 2 else y_g2
                        kk = k if k < KT // 2 else k - KT // 2
                        for half in range(2):
                            rb = rp * 2 + half
                            nc.sync.dma_start(
                                r(yg_sb[:, half * SEQB:(half + 1) * SEQB]),
                                r(ysrc[rb * H // 2 + kk * 128:
                                       rb * H // 2 + (kk + 1) * 128, :]))
                        for mt in range(MT):
                            nc.tensor.matmul(
                                mps[mt][:],
                                r(qwT_sb[:, k * QKVR + mt * 128:
                                         k * QKVR + (mt + 1) * 128]),
                                r(yg_sb[:]),
                                start=(k == 0), stop=(k == KT - 1))
                    for mt in range(MT):
                        nc.vector.tensor_scalar_add(
                            r(mixT[:, mt * SQ + rp * 512: mt * SQ + (rp + 1) * 512]),
                            mps[mt][:], qb_sb[:, mt:mt + 1])

            # ---- stage C: rotations + attention per head ----
            ctx_in = [dram.tile([N_CORES, HD, SEQB], BF16, name=f"ctxin{hl}")
                      for hl in range(HPC)]
            ctx_a = [dram.tile([N_CORES, HD, SEQB], BF16, name=f"ctxa{hl}")
                     for hl in range(HPC)]
            # prefetch tsr (bf16) during stage C
            for kt in range(KT):
                nc.sync.dma_start(
                    tsrb_sb[:, kt * H:(kt + 1) * H],
                    tsr_g[kt * 128:(kt + 1) * 128, :])
            with (
                tc.tile_pool(name="sC", bufs=1) as sC,
                tc.tile_pool(name="pC", bufs=2, space="PSUM") as pC,
                tc.tile_pool(name="sD", bufs=2) as sD,
            ):
                for hl in range(HPC):
                    qrow, krow, vrow = hl * 3, hl * 3 + 1, hl * 3 + 2
                    qrotT = sC.tile([128, SQ], F32, tag="qrot", bufs=2)
                    krotT = sC.tile([128, SQ], F32, tag="krot", bufs=2)
                    vrot = sC.tile([128, SQ], BF16, tag="vrot", bufs=2)
                    for sc in range(4):
                        rp1 = pC.tile([128, 512], F32, tag="rotp")
                        nc.tensor.matmul(
                            rp1[:], r(sqk_sb[:, hl * HD:(hl + 1) * HD]),
                            r(mixT[:, qrow * SQ + sc * 512:
                                   qrow * SQ + (sc + 1) * 512]),
                            start=True, stop=True)
                        nc.scalar.activation(
                            r(qrotT[:, sc * 512:(sc + 1) * 512]), rp1[:],
                            mybir.ActivationFunctionType.Copy, scale=SCALE)
                        rp2 = pC.tile([128, 512], F32, tag="rotp")
                        nc.tensor.matmul(
                            rp2[:], r(sqk_sb[:, hl * HD:(hl + 1) * HD]),
                            r(mixT[:, krow * SQ + sc * 512:
                                   krow * SQ + (sc + 1) * 512]),
                            start=True, stop=True)
                        nc.vector.tensor_copy(
                            r(krotT[:, sc * 512:(sc + 1) * 512]), rp2[:])
                    for st in range(KT):
                        vp = pC.tile([128, 128], F32, tag="rotp")
                        nc.tensor.matmul(
                            vp[:],
                            r(mixT[:, vrow * SQ + st * 128:
                                   vrow * SQ + (st + 1) * 128]),
                            r(svl_sb[:, hl * HD:(hl + 1) * HD]),
                            start=True, stop=True)
                        nc.vector.tensor_copy(vrot[:, st * 128:(st + 1) * 128],
                                              vp[:])

                    ctxT_sb = sC.tile([128, SQ], BF16, tag="ctxT", bufs=2)
                    for rb in range(4):
                        ncb = 4 * (rb + 1) if causal else KT
                        ctp = pC.tile([128, 512], F32, tag="ctp")
                        lp = pC.tile([1, 512], F32, tag="lp", bufs=1)
                        for cb in range(ncb):
                            sp = pC.tile([128, 512], F32, tag="sp")
                            nc.tensor.matmul(
                                sp[:], r(krotT[:, cb * 128:(cb + 1) * 128]),
                                r(qrotT[:, rb * 512:(rb + 1) * 512]),
                                start=True, stop=True)
                            if causal and cb >= 4 * rb:
                                o = 384 - (cb * 128 - rb * 512)
                                nc.vector.tensor_tensor(
                                    sp[:], sp[:], tb_sb[:, o:o + 512],
                                    mybir.AluOpType.add)
                            pT = sD.tile([128, 512], BF16, tag="pT", bufs=3)
                            nc.scalar.activation(
                                pT[:], sp[:], mybir.ActivationFunctionType.Exp)
                            nc.tensor.matmul(
                                ctp[:], vrot[:, cb * 128:(cb + 1) * 128], pT[:],
                                start=(cb == 0), stop=(cb == ncb - 1))
                            nc.tensor.matmul(
                                lp[:], onesb_sb[:, 0:1], pT[:],
                                start=(cb == 0), stop=(cb == ncb - 1))
                        linv = sD.tile([1, 512], F32, tag="linv")
                        nc.vector.reciprocal(r(linv[:]), lp[:])
                        lbp = pC.tile([128, 512], F32, tag="lbp", bufs=1)
                        nc.tensor.matmul(lbp[:], r(ones_sb[0:1, :]), r(linv[:]),
                                         start=True, stop=True)
                        lb_sb = sD.tile([128, 512], F32, tag="lb")
                        nc.vector.tensor_copy(lb_sb[:], lbp[:])
                        nc.vector.tensor_tensor(
                            ctxT_sb[:, rb * 512:(rb + 1) * 512], ctp[:], lb_sb[:],
                            mybir.AluOpType.mult)
                    for b in range(N_CORES):
                        nc.sync.dma_start(
                            ctx_in[hl][b, :, :],
                            ctxT_sb[:, b * SEQB:(b + 1) * SEQB])
                    nc.gpsimd.collective_compute(
                        "AllToAll", mybir.AluOpType.bypass, replica_groups=rg,
                        ins=[ctx_in[hl][:].opt()], outs=[ctx_a[hl][:].opt()])

            # ---- stage E: out = ctx_myblock @ tsr + dense_b (bf16) ----
            with (
                tc.tile_pool(name="sE", bufs=2) as sE,
                tc.tile_pool(name="pE", bufs=4, space="PSUM") as pE,
            ):
                ctxa_sb = sE.tile([128, KT * SEQB], BF16, tag="ctxa", bufs=1)
                for kt in range(KT):
                    nc.sync.dma_start(
                        ctxa_sb[:, kt * SEQB:(kt + 1) * SEQB],
                        ctx_a[kt % HPC][kt // HPC, :, :])
                for n in range(4):
                    for mt in range(2):
                        op = pE.tile([128, 512], F32, tag="op")
                        for kt in range(KT):
                            nc.tensor.matmul(
                                op[:],
                                ctxa_sb[:, kt * SEQB + mt * 128:
                                        kt * SEQB + (mt + 1) * 128],
                                tsrb_sb[:, kt * H + n * 512:
                                        kt * H + (n + 1) * 512],
                                start=(kt == 0), stop=(kt == KT - 1))
                        os_ = sE.tile([128, 512], F32, tag="os")
                        nc.vector.tensor_tensor(
                            os_[:], op[:], bb_sb[:, n * 512:(n + 1) * 512],
                            mybir.AluOpType.add)
                        nc.sync.dma_start(
                            out[mt * 128:(mt + 1) * 128, n * 512:(n + 1) * 512],
                            os_[:])
            mid.release()
    nc.compile()
    return nc


_CAUSAL_MASK = None


def _is_causal(mask):
    global _CAUSAL_MASK
    m = np.asarray(mask).reshape(SQ, SQ)
    if _CAUSAL_MASK is None:
        _CAUSAL_MASK = np.triu(np.ones((SQ, SQ), dtype=bool), k=1)
    return np.array_equal(m, _CAUSAL_MASK)


def make_in_maps(inputs):
    hidden_states = np.asarray(inputs["hidden_states"], np.float32)
    qkv_w = np.asarray(inputs["qkv_w"], np.float32)
    qkv_b = np.asarray(inputs["qkv_b"], np.float32)
    svd_token = np.ascontiguousarray(np.asarray(inputs["svd_token"], np.float32))
    svd_tokenT = np.ascontiguousarray(svd_token.T)
    svd_qk = np.asarray(inputs["svd_qk"], np.float32)
    svd_vlin = np.asarray(inputs["svd_vlin"], np.float32)
    dense_w = np.asarray(inputs["dense_w"], np.float32)
    dense_b = np.asarray(inputs["dense_b"], np.float32)

    hTf = np.ascontiguousarray(hidden_states[:, 0, :].T)        # [H, SQ]
    qwTf = np.ascontiguousarray(qkv_w.T)                        # [H, 3H]
    in_maps = []
    for c in range(N_CORES):
        h0 = c * HPC
        rows = slice(c * QKVR, (c + 1) * QKVR)
        in_maps.append({
            "hT": np.ascontiguousarray(hTf[:, c * SEQB:(c + 1) * SEQB]),
            "qwT": np.ascontiguousarray(qwTf[:, rows]),
            "qbT": np.ascontiguousarray(qkv_b[rows].reshape(MT, 128).T),
            "stok": svd_token,
            "stokT": svd_tokenT,
            "sqk": np.ascontiguousarray(svd_qk[h0:h0 + HPC]),
            "svl": np.ascontiguousarray(svd_vlin[h0:h0 + HPC]),
            "dw": np.ascontiguousarray(dense_w[h0:h0 + HPC]),
            "dbB": np.ascontiguousarray(dense_b.reshape(1, H)),
        })
    return in_maps


def kernel(hidden_states, attention_mask, qkv_w, qkv_b, svd_token,
           svd_qk, svd_vlin, dense_w, dense_b):
    causal = _is_causal(attention_mask)
    if not causal:
        assert not np.asarray(attention_mask).any(), \
            "kernel supports causal or empty attention_mask"

    nc = build(causal=causal)
    in_maps = make_in_maps({
        "hidden_states": hidden_states, "qkv_w": qkv_w, "qkv_b": qkv_b,
        "svd_token": svd_token, "svd_qk": svd_qk, "svd_vlin": svd_vlin,
        "dense_w": dense_w, "dense_b": dense_b,
    })
    res = bass_utils.run_bass_kernel_spmd(
        nc, in_maps, core_ids=list(range(N_CORES)), trace=False)
    full = np.concatenate([res.results[c]["out"] for c in range(N_CORES)], axis=0)
    return full.reshape(SQ, 1, H)


# revision 16
# speedup vs baseline: 1.2599x; 1.0781x over previous
"""Trainium2 8-core tensor-parallel sparse-attention kernel (Bass/Tile).

Reference (SQ=2048, B=1, H=2048, NH=16, HD=128):
    x = hidden[:,0,:] @ svd_token
    w = qkv_w @ svd_token;  mixed = x @ w.T + qkv_b
    per head h: q,k rotated by svd_qk[h], v by svd_vlin[h]
    scores = qr @ kr.T / sqrt(128) causal-masked, softmax
    ctx = probs @ vr;  tsr[h] = svd_vlin[h].T @ dense_w[h]
    out = ctx @ tsr + dense_b

Key identity: mixed = x @ (qkv_w @ st).T = (x @ st.T) @ qkv_w.T = y @ qkv_w.T
 -> compute y seq-sharded (2.15 GF/core) instead of w head-sharded (6.4).

Per-core pipeline (TP over heads, 2 heads/core):
  pass1: x^T_c = st-panels.T @ hidden^T_c            (f32r)
  pass2: y^T_c = stT-panels.T @ x^T_c  -> AllGather  (f32r)
  fill:  tsr shard -> AllGather; dense_b broadcast   (overlaps AG_y)
  B2:    mixed^T = qkv_w-shard @ y^T, rank-pairs N=512 (f32r)
  C:     per head: q/k rot f32r, v rot bf16; scores^T f32r; raw exp
         (scores bounded ~15); P@V + row-sum ones-matmul in bf16;
         normalize post-PV; ctx^T -> per-head AllToAll (bf16)
  E:     out = ctx_myblock @ tsr + dense_b, bf16 (tsr cast-prefetched
         to SBUF during C)
Host only shards inputs / concatenates the 8 output row-blocks.
"""
import math

import ml_dtypes
import numpy as np

import concourse.bass as bass
import concourse.mybir as mybir
import concourse.bacc as bacc
import concourse.tile as tile
from concourse import bass_utils

N_CORES = 8
SQ = 2048
H = 2048
NH = 16
HD = 128
HPC = NH // N_CORES          # heads per core = 2
QKVR = HPC * 3 * HD          # qkv rows per core = 768
SEQB = SQ // N_CORES         # seq block per core = 256
KT = H // 128                # 128-tiles over hidden = 16
MT = QKVR // 128             # qkv row tiles = 6
F32 = mybir.dt.float32
F32R = mybir.dt.float32r
BF16 = mybir.dt.bfloat16
NEG = -30000.0
SCALE = 1.0 / math.sqrt(HD)


def r(ap):
    return ap.bitcast(F32R)


def build(causal=True):
    nc = bacc.Bacc("TRN2", target_bir_lowering=False, debug=False,
                   num_devices=N_CORES)

    hT = nc.dram_tensor("hT", [H, SEQB], F32, kind="ExternalInput")
    qwT = nc.dram_tensor("qwT", [H, QKVR], BF16, kind="ExternalInput")
    qbT = nc.dram_tensor("qbT", [128, MT], F32, kind="ExternalInput")
    stok = nc.dram_tensor("stok", [H, H], F32, kind="ExternalInput")
    stokT = nc.dram_tensor("stokT", [H, H], F32, kind="ExternalInput")
    sqk = nc.dram_tensor("sqk", [HPC, HD, HD], F32, kind="ExternalInput")
    svl = nc.dram_tensor("svl", [HPC, HD, HD], F32, kind="ExternalInput")
    dw = nc.dram_tensor("dw", [HPC, HD, H], F32, kind="ExternalInput")
    dbB = nc.dram_tensor("dbB", [1, H], F32, kind="ExternalInput")
    out = nc.dram_tensor("out", [SEQB, H], F32, kind="ExternalOutput")

    ones_dram = nc.inline_tensor(np.ones((128, 128), np.float32), name="ones_c")
    id_dram = nc.inline_tensor(np.eye(128, dtype=np.float32), name="id_c")
    tb_np = np.where(
        np.arange(128)[:, None] > np.arange(896)[None, :] - 384, NEG, 0.0
    ).astype(np.float32)
    tb_dram = nc.inline_tensor(tb_np, name="triband_c")

    rg = [list(range(N_CORES))]

    with tile.TileContext(nc) as tc:
        with (
            nc.allow_low_precision(reason="f32r/bf16 for full-rate PE"),
            tc.tile_pool(name="pers", bufs=1) as pers,
            tc.tile_pool(name="dram", bufs=1, space="DRAM") as dram,
        ):
            # ---- persistent constants ----
            ones_sb = pers.tile([128, 128], F32)
            onesb_sb = pers.tile([128, 128], BF16)
            tb_sb = pers.tile([128, 896], F32)
            id_sb = pers.tile([128, 128], F32)
            nc.sync.dma_start(id_sb[:], id_dram[:])
            nc.sync.dma_start(r(ones_sb[:]), r(ones_dram[:]))
            nc.gpsimd.dma_start(onesb_sb[:], ones_dram[:])   # cast dma
            nc.sync.dma_start(tb_sb[:], tb_dram[:])
            qb_sb = pers.tile([128, MT], F32)
            nc.sync.dma_start(qb_sb[:], qbT[:])
            sqk_sb = pers.tile([128, HPC * HD], F32)
            svl_sb = pers.tile([128, HPC * HD], F32)
            for hl in range(HPC):
                nc.sync.dma_start(r(sqk_sb[:, hl * HD:(hl + 1) * HD]), r(sqk[hl]))
                nc.sync.dma_start(r(svl_sb[:, hl * HD:(hl + 1) * HD]), r(svl[hl]))
            db_sb = pers.tile([1, H], F32)
            nc.sync.dma_start(r(db_sb[:]), r(dbB[:]))

            y_in1 = dram.tile([H // 2, SEQB], BF16)
            y_in2 = dram.tile([H // 2, SEQB], BF16)
            y_g1 = dram.tile([N_CORES * H // 2, SEQB], BF16, addr_space="Shared")
            y_g2 = dram.tile([N_CORES * H // 2, SEQB], BF16, addr_space="Shared")
            tsr_in = dram.tile([HPC * HD, H], BF16)
            tsr_g = dram.tile([NH * HD, H], BF16, addr_space="Shared")

            # ---- pass1/pass2 (flipped): stationary = hT/xT tiles,
            #      moving = svd_token row-panels at N=512 (LDW amortized) ----
            with (
                tc.tile_pool(name="sA", bufs=2) as sA,
                tc.tile_pool(name="pA", bufs=8, space="PSUM") as pA,
            ):
                hT_sb = sA.tile([128, KT * SEQB], F32, tag="hTt", bufs=1)
                nc.sync.dma_start(
                    r(hT_sb[:].rearrange("p (k s) -> p k s", k=KT)),
                    r(hT.rearrange("(k p) s -> p k s", p=128)))
                # pass1: x[s, j] = sum_k hT[k, s].T @ stok[k, j]
                xps = [pA.tile([128, 512], F32, tag="acc", name=f"xps{i}", bufs=8)
                       for i in range(8)]
                for k in range(KT):
                    srow = sA.tile([128, H], F32, tag="srow", bufs=3)
                    nc.sync.dma_start(r(srow[:]), r(stok[k * 128:(k + 1) * 128, :]))
                    for st in range(2):
                        for jc in range(4):
                            nc.tensor.matmul(
                                xps[st * 4 + jc][:],
                                r(hT_sb[:, k * SEQB + st * 128:
                                        k * SEQB + (st + 1) * 128]),
                                r(srow[:, jc * 512:(jc + 1) * 512]),
                                start=(k == 0), stop=(k == KT - 1))
                x_sb = sA.tile([128, 2 * H], F32, tag="xsb", bufs=1)
                for i in range(8):
                    nc.vector.tensor_copy(
                        x_sb[:, i * 512:(i + 1) * 512], xps[i][:])
                # transpose x -> xT (m on partitions)
                xT_sb = sA.tile([128, KT * SEQB], F32, tag="xTt", bufs=1)
                for m in range(KT):
                    for st in range(2):
                        tp2 = pA.tile([128, 128], F32, tag="acc",
                                      name=f"tpx{m}_{st}", bufs=8)
                        nc.tensor.transpose(
                            tp2[:],
                            x_sb[:, st * H + m * 128: st * H + (m + 1) * 128],
                            id_sb[:])
                        nc.vector.tensor_copy(
                            r(xT_sb[:, m * SEQB + st * 128:
                                    m * SEQB + (st + 1) * 128]), tp2[:])
                # pass2: y[s, j2] = sum_m xT[m, s].T @ stokT[m, j2]
                yps = [pA.tile([128, 512], F32, tag="acc", name=f"yps{i}", bufs=8)
                       for i in range(8)]
                for m in range(KT):
                    srow2 = sA.tile([128, H], F32, tag="srow", bufs=3)
                    nc.sync.dma_start(r(srow2[:]),
                                      r(stokT[m * 128:(m + 1) * 128, :]))
                    for st in range(2):
                        for jc in range(4):
                            nc.tensor.matmul(
                                yps[st * 4 + jc][:],
                                r(xT_sb[:, m * SEQB + st * 128:
                                        m * SEQB + (st + 1) * 128]),
                                r(srow2[:, jc * 512:(jc + 1) * 512]),
                                start=(m == 0), stop=(m == KT - 1))
                y_sb = sA.tile([128, 2 * H], F32, tag="xsb", bufs=1)
                for i in range(8):
                    nc.vector.tensor_copy(
                        y_sb[:, i * 512:(i + 1) * 512], yps[i][:])
                # transpose y -> yT tiles, stage + DMA out, AGs per half
                for j2 in range(KT):
                    ystg = sA.tile([128, SEQB], BF16, tag="ystg", bufs=4)
                    for st in range(2):
                        tp3 = pA.tile([128, 128], F32, tag="acc",
                                      name=f"tpy{j2}_{st}", bufs=8)
                        nc.tensor.transpose(
                            tp3[:],
                            y_sb[:, st * H + j2 * 128: st * H + (j2 + 1) * 128],
                            id_sb[:])
                        nc.vector.tensor_copy(
                            ystg[:, st * 128:(st + 1) * 128], tp3[:])
                    if j2 < KT // 2:
                        nc.sync.dma_start(
                            y_in1[j2 * 128:(j2 + 1) * 128, :], ystg[:])
                    else:
                        nc.sync.dma_start(
                            y_in2[(j2 - KT // 2) * 128:(j2 - KT // 2 + 1) * 128, :],
                            ystg[:])
                    if j2 == KT // 2 - 1:
                        nc.gpsimd.collective_compute(
                            "AllGather", mybir.AluOpType.bypass, replica_groups=rg,
                            ins=[y_in1[:].opt()], outs=[y_g1[:].opt()])
            nc.gpsimd.collective_compute(
                "AllGather", mybir.AluOpType.bypass, replica_groups=rg,
                ins=[y_in2[:].opt()], outs=[y_g2[:].opt()])

            # ---- tsr shard (bf16) + AG_tsr + dense_b broadcast ----
            bb_sb = pers.tile([128, H], BF16)
            with (
                tc.tile_pool(name="s0", bufs=2) as s0,
                tc.tile_pool(name="p0", bufs=2, space="PSUM") as p0,
            ):
                dw_sb = s0.tile([128, HPC * H], F32, tag="dwt", bufs=1)
                for hl in range(HPC):
                    nc.sync.dma_start(r(dw_sb[:, hl * H:(hl + 1) * H]), r(dw[hl]))
                for hl in range(HPC):
                    tsr_sb = s0.tile([128, H], BF16, tag="tsr")
                    for n in range(4):
                        tp = p0.tile([128, 512], F32, tag="t0p")
                        nc.tensor.matmul(
                            tp[:], r(svl_sb[:, hl * HD:(hl + 1) * HD]),
                            r(dw_sb[:, hl * H + n * 512: hl * H + (n + 1) * 512]),
                            start=True, stop=True)
                        nc.vector.tensor_copy(
                            tsr_sb[:, n * 512:(n + 1) * 512], tp[:])
                    nc.sync.dma_start(tsr_in[hl * HD:(hl + 1) * HD, :], tsr_sb[:])
                for n in range(4):
                    bp = p0.tile([128, 512], F32, tag="t0p")
                    nc.tensor.matmul(bp[:], r(ones_sb[0:1, :]),
                                     r(db_sb[:, n * 512:(n + 1) * 512]),
                                     start=True, stop=True)
                    nc.vector.tensor_copy(bb_sb[:, n * 512:(n + 1) * 512], bp[:])
            nc.gpsimd.collective_compute(
                "AllGather", mybir.AluOpType.bypass, replica_groups=rg,
                ins=[tsr_in[:].opt()], outs=[tsr_g[:].opt()])

            # ---- B2: mixed^T = qw_shard @ y^T, rank-pairs (N=512) ----
            mid = tc.alloc_tile_pool(name="mid", bufs=1)
            mixT = mid.tile([128, MT * SQ], F32, name="mixT")
            tsrb_sb = mid.tile([128, KT * H], BF16, name="tsrb_sb")
            with (
                tc.tile_pool(name="sB", bufs=2) as sB,
                tc.tile_pool(name="pB", bufs=6, space="PSUM") as pB,
            ):
                qwT_sb = sB.tile([128, KT * QKVR], BF16, tag="qwT", bufs=1)
                nc.sync.dma_start(
                    qwT_sb[:].rearrange("p (k q) -> p k q", k=KT),
                    qwT.rearrange("(k p) q -> p k q", p=128))
                for rp in range(N_CORES // 2):
                    mps = [pB.tile([128, 512], F32, tag="mp", name=f"mp{rp}_{i}") for i in range(MT)]
                    for k in range(KT):
                        yg_sb = sB.tile([128, 512], BF16, tag="yg", bufs=4)
                        ysrc = y_g1 if k < KT // 2 else y_g2
                        kk = k if k < KT // 2 else k - KT // 2
                        for half in range(2):
                            rb = rp * 2 + half
                            nc.sync.dma_start(
                                yg_sb[:, half * SEQB:(half + 1) * SEQB],
                                ysrc[rb * H // 2 + kk * 128:
                                     rb * H // 2 + (kk + 1) * 128, :])
                        for mt in range(MT):
                            nc.tensor.matmul(
                                mps[mt][:],
                                qwT_sb[:, k * QKVR + mt * 128:
                                       k * QKVR + (mt + 1) * 128],
                                yg_sb[:],
                                start=(k == 0), stop=(k == KT - 1))
                    for mt in range(MT):
                        nc.vector.tensor_scalar_add(
                            r(mixT[:, mt * SQ + rp * 512: mt * SQ + (rp + 1) * 512]),
                            mps[mt][:], qb_sb[:, mt:mt + 1])

            # ---- stage C: rotations + attention per head ----
            ctx_in = [dram.tile([N_CORES, HD, SEQB], BF16, name=f"ctxin{hl}")
                      for hl in range(HPC)]
            ctx_a = [dram.tile([N_CORES, HD, SEQB], BF16, name=f"ctxa{hl}")
                     for hl in range(HPC)]
            # prefetch tsr (bf16) during stage C
            for kt in range(KT):
                nc.sync.dma_start(
                    tsrb_sb[:, kt * H:(kt + 1) * H],
                    tsr_g[kt * 128:(kt + 1) * 128, :])
            with (
                tc.tile_pool(name="sC", bufs=1) as sC,
                tc.tile_pool(name="pC", bufs=2, space="PSUM") as pC,
                tc.tile_pool(name="sD", bufs=2) as sD,
            ):
                for hl in range(HPC):
                    qrow, krow, vrow = hl * 3, hl * 3 + 1, hl * 3 + 2
                    qrotT = sC.tile([128, SQ], F32, tag="qrot", bufs=2)
                    krotT = sC.tile([128, SQ], F32, tag="krot", bufs=2)
                    vrot = sC.tile([128, SQ], BF16, tag="vrot", bufs=2)
                    for sc in range(4):
                        rp1 = pC.tile([128, 512], F32, tag="rotp")
                        nc.tensor.matmul(
                            rp1[:], r(sqk_sb[:, hl * HD:(hl + 1) * HD]),
                            r(mixT[:, qrow * SQ + sc * 512:
                                   qrow * SQ + (sc + 1) * 512]),
                            start=True, stop=True)
                        nc.scalar.activation(
                            r(qrotT[:, sc * 512:(sc + 1) * 512]), rp1[:],
                            mybir.ActivationFunctionType.Copy, scale=SCALE)
                        rp2 = pC.tile([128, 512], F32, tag="rotp")
                        nc.tensor.matmul(
                            rp2[:], r(sqk_sb[:, hl * HD:(hl + 1) * HD]),
                            r(mixT[:, krow * SQ + sc * 512:
                                   krow * SQ + (sc + 1) * 512]),
                            start=True, stop=True)
                        nc.vector.tensor_copy(
                            r(krotT[:, sc * 512:(sc + 1) * 512]), rp2[:])
                    for st in range(KT):
                        vp = pC.tile([128, 128], F32, tag="rotp")
                        nc.tensor.matmul(
                            vp[:],
                            r(mixT[:, vrow * SQ + st * 128:
                                   vrow * SQ + (st + 1) * 128]),
                            r(svl_sb[:, hl * HD:(hl + 1) * HD]),
                            start=True, stop=True)
                        nc.vector.tensor_copy(vrot[:, st * 128:(st + 1) * 128],
                                              vp[:])

                    ctxT_sb = sC.tile([128, SQ], BF16, tag="ctxT", bufs=2)
                    for rb in range(4):
                        ncb = 4 * (rb + 1) if causal else KT
                        ctp = pC.tile([128, 512], F32, tag="ctp")
                        lp = pC.tile([1, 512], F32, tag="lp", bufs=1)
                        for cb in range(ncb):
                            sp = pC.tile([128, 512], F32, tag="sp")
                            nc.tensor.matmul(
                                sp[:], r(krotT[:, cb * 128:(cb + 1) * 128]),
                                r(qrotT[:, rb * 512:(rb + 1) * 512]),
                                start=True, stop=True)
                            if causal and cb >= 4 * rb:
                                o = 384 - (cb * 128 - rb * 512)
                                nc.vector.tensor_tensor(
                                    sp[:], sp[:], tb_sb[:, o:o + 512],
                                    mybir.AluOpType.add)
                            pT = sD.tile([128, 512], BF16, tag="pT", bufs=3)
                            nc.scalar.activation(
                                pT[:], sp[:], mybir.ActivationFunctionType.Exp)
                            nc.tensor.matmul(
                                ctp[:], vrot[:, cb * 128:(cb + 1) * 128], pT[:],
                                start=(cb == 0), stop=(cb == ncb - 1))
                            nc.tensor.matmul(
                                lp[:], onesb_sb[:, 0:1], pT[:],
                                start=(cb == 0), stop=(cb == ncb - 1))
                        linv = sD.tile([1, 512], F32, tag="linv")
                        nc.vector.reciprocal(r(linv[:]), lp[:])
                        lbp = pC.tile([128, 512], F32, tag="lbp", bufs=1)
                        nc.tensor.matmul(lbp[:], r(ones_sb[0:1, :]), r(linv[:]),
                                         start=True, stop=True)
                        lb_sb = sD.tile([128, 512], F32, tag="lb")
                        nc.vector.tensor_copy(lb_sb[:], lbp[:])
                        nc.vector.tensor_tensor(
                            ctxT_sb[:, rb * 512:(rb + 1) * 512], ctp[:], lb_sb[:],
                            mybir.AluOpType.mult)
                    for b in range(N_CORES):
                        nc.sync.dma_start(
                            ctx_in[hl][b, :, :],
                            ctxT_sb[:, b * SEQB:(b + 1) * SEQB])
                    nc.gpsimd.collective_compute(
                        "AllToAll", mybir.AluOpType.bypass, replica_groups=rg,
                        ins=[ctx_in[hl][:].opt()], outs=[ctx_a[hl][:].opt()])

            # ---- stage E: out = ctx_myblock @ tsr + dense_b (bf16) ----
            with (
                tc.tile_pool(name="sE", bufs=2) as sE,
                tc.tile_pool(name="pE", bufs=4, space="PSUM") as pE,
            ):
                ctxa_sb = sE.tile([128, KT * SEQB], BF16, tag="ctxa", bufs=1)
                for kt in range(KT):
                    nc.sync.dma_start(
                        ctxa_sb[:, kt * SEQB:(kt + 1) * SEQB],
                        ctx_a[kt % HPC][kt // HPC, :, :])
                for n in range(4):
                    for mt in range(2):
                        op = pE.tile([128, 512], F32, tag="op")
                        for kt in range(KT):
                            nc.tensor.matmul(
                                op[:],
                                ctxa_sb[:, kt * SEQB + mt * 128:
                                        kt * SEQB + (mt + 1) * 128],
                                tsrb_sb[:, kt * H + n * 512:
                                        kt * H + (n + 1) * 512],
                                start=(kt == 0), stop=(kt == KT - 1))
                        os_ = sE.tile([128, 512], F32, tag="os")
                        nc.vector.tensor_tensor(
                            os_[:], op[:], bb_sb[:, n * 512:(n + 1) * 512],
                            mybir.AluOpType.add)
                        nc.sync.dma_start(
                            out[mt * 128:(mt + 1) * 128, n * 512:(n + 1) * 512],
                            os_[:])
            mid.release()
    nc.compile()
    return nc


_CAUSAL_MASK = None


def _is_causal(mask):
    global _CAUSAL_MASK
    m = np.asarray(mask).reshape(SQ, SQ)
    if _CAUSAL_MASK is None:
        _CAUSAL_MASK = np.triu(np.ones((SQ, SQ), dtype=bool), k=1)
    return np.array_equal(m, _CAUSAL_MASK)


def make_in_maps(inputs):
    hidden_states = np.asarray(inputs["hidden_states"], np.float32)
    qkv_w = np.asarray(inputs["qkv_w"], np.float32)
    qkv_b = np.asarray(inputs["qkv_b"], np.float32)
    svd_token = np.ascontiguousarray(np.asarray(inputs["svd_token"], np.float32))
    svd_tokenT = np.ascontiguousarray(svd_token.T)
    svd_qk = np.asarray(inputs["svd_qk"], np.float32)
    svd_vlin = np.asarray(inputs["svd_vlin"], np.float32)
    dense_w = np.asarray(inputs["dense_w"], np.float32)
    dense_b = np.asarray(inputs["dense_b"], np.float32)

    hTf = np.ascontiguousarray(hidden_states[:, 0, :].T)        # [H, SQ]
    qwTf = np.ascontiguousarray(qkv_w.T)                        # [H, 3H]
    in_maps = []
    for c in range(N_CORES):
        h0 = c * HPC
        rows = slice(c * QKVR, (c + 1) * QKVR)
        in_maps.append({
            "hT": np.ascontiguousarray(hTf[:, c * SEQB:(c + 1) * SEQB]),
            "qwT": np.ascontiguousarray(qwTf[:, rows]).astype(ml_dtypes.bfloat16),
            "qbT": np.ascontiguousarray(qkv_b[rows].reshape(MT, 128).T),
            "stok": svd_token,
            "stokT": svd_tokenT,
            "sqk": np.ascontiguousarray(svd_qk[h0:h0 + HPC]),
            "svl": np.ascontiguousarray(svd_vlin[h0:h0 + HPC]),
            "dw": np.ascontiguousarray(dense_w[h0:h0 + HPC]),
            "dbB": np.ascontiguousarray(dense_b.reshape(1, H)),
        })
    return in_maps


def kernel(hidden_states, attention_mask, qkv_w, qkv_b, svd_token,
           svd_qk, svd_vlin, dense_w, dense_b):
    causal = _is_causal(attention_mask)
    if not causal:
        assert not np.asarray(attention_mask).any(), \
            "kernel supports causal or empty attention_mask"

    nc = build(causal=causal)
    in_maps = make_in_maps({
        "hidden_states": hidden_states, "qkv_w": qkv_w, "qkv_b": qkv_b,
        "svd_token": svd_token, "svd_qk": svd_qk, "svd_vlin": svd_vlin,
        "dense_w": dense_w, "dense_b": dense_b,
    })
    res = bass_utils.run_bass_kernel_spmd(
        nc, in_maps, core_ids=list(range(N_CORES)), trace=False)
    full = np.concatenate([res.results[c]["out"] for c in range(N_CORES)], axis=0)
    return full.reshape(SQ, 1, H)
